# revision 1
# baseline (speedup 1.0000x reference)
"""Trainium2 Bass kernel: 12-head MHA (B=2, S=4096, D=768), sharded over 8 cores.

Sharding: core c -> batch b = c//4, head group g = c%4 (3 heads of 64 dims each).
Each core computes a partial output [S, 768] = ctx_g @ Wo_g^T for its 3 heads;
the host sums the 4 partials per batch and adds bo.

Device dataflow (per core, all matmul moving dims bf16, PSUM f32 accumulate):
  - QKV projections from pre-transposed x^T (sent bf16 from host).
    Q^T/K^T stored [128, S] per head with rows 0-63 = data, 64-127 = duplicate
    so score matmuls (contraction d=64) can pair consecutive k-tiles onto the
    two 64-row PE array tiles (tile_position rows 0 / 64) and run concurrently.
  - Scores S^T[k, q] = K^T.T @ Q^T, PSUM; exp via ScalarE with fused 1/8 scale
    (softmax max-subtraction skipped: |scores/8| < ~3 for this distribution).
  - Context^T = V_aug^T @ expS^T with V_aug = [V | ones]: row 64 of the result
    is the softmax denominator. Contraction (k=128 per tile) split into two
    64-row array tiles -> two PSUM accumulators, summed on VectorE.
  - Normalize by broadcasted reciprocal of the denominator row, then output
    projection out[q, m] = ctxN^T.T @ Wo^T accumulated over the 3 heads.
"""

import sys

for _p in ("/opt/trn_rl_repo",):
    if _p not in sys.path:
        sys.path.insert(0, _p)

import numpy as np
import ml_dtypes

BF16 = ml_dtypes.bfloat16

D = 768
DG = 192          # head dims per core (3 heads x 64)
DK = 64
NH = 3            # heads per core
CKT = D // 128    # contraction k-tiles for projections


def build_nc(S, debug_dumps=False):
    import concourse.mybir as mybir
    import concourse.tile as tile
    import concourse.bacc as bacc
    from contextlib import ExitStack

    dt = mybir.dt
    f32, bf = dt.float32, dt.bfloat16
    ALU = mybir.AluOpType
    ACTF = mybir.ActivationFunctionType

    NKT = S // 128        # seq tiles (k dimension of attention)
    NJ = S // 512         # q chunks
    VW = NH * 65          # V_aug cols per seq tile

    nc = bacc.Bacc("TRN2", target_bir_lowering=False, debug=False)

    xt_d = nc.dram_tensor("xt", [D, S], bf, kind="ExternalInput").ap()
    wqt_d = nc.dram_tensor("wqt", [D, DG], bf, kind="ExternalInput").ap()
    wkt_d = nc.dram_tensor("wkt", [D, DG], bf, kind="ExternalInput").ap()
    wvt_d = nc.dram_tensor("wvt", [D, DG], bf, kind="ExternalInput").ap()
    wot_d = nc.dram_tensor("wot", [DG, D], bf, kind="ExternalInput").ap()
    bq_d = nc.dram_tensor("bq", [DG, 1], f32, kind="ExternalInput").ap()
    bk_d = nc.dram_tensor("bk", [DG, 1], f32, kind="ExternalInput").ap()
    bv_d = nc.dram_tensor("bv", [1, DG], f32, kind="ExternalInput").ap()
    out_d = nc.dram_tensor("out", [S, D], f32, kind="ExternalOutput").ap()
    NKT = S // 128
    dbg = {}
    if debug_dumps:
        for nm, shp, dty in [("qt0", [128, S], bf), ("kt0", [128, S], bf),
                             ("v", [128, NKT * NH * 65], bf),
                             ("expS", [128, NKT * 512], bf),
                             ("s2", [65, 512], f32), ("ri", [64, 512], f32),
                             ("ctxN0", [64, S], bf)]:
            dbg[nm] = nc.dram_tensor(f"dbg_{nm}", shp, dty,
                                     kind="ExternalOutput").ap()

    with tile.TileContext(nc) as tc, ExitStack() as ctx:
        wp = ctx.enter_context(tc.tile_pool(name="wp", bufs=1))
        ps = ctx.enter_context(tc.tile_pool(name="ps", bufs=1, space="PSUM"))

        # ---- persistent SBUF ----
        qt = [wp.tile([128, S], bf, name=f"qt{h}") for h in range(NH)]
        kt_ = [wp.tile([128, S], bf, name=f"kt{h}") for h in range(NH)]
        v_sb = wp.tile([128, NKT * VW], bf, name="v_sb")
        expS = wp.tile([128, NKT * 512], bf, name="expS")
        ctxN = [wp.tile([64, S], bf, name=f"ctxN{h}") for h in range(NH)]
        wo_sb = [wp.tile([64, D], bf, name=f"wo{h}") for h in range(NH)]
        for h in range(NH):
            nc.sync.dma_start(wo_sb[h][:], wot_d[h * 64:(h + 1) * 64, :])
        # V_aug ones columns: set whole tile to 1.0 first, drains overwrite data
        nc.gpsimd.memset(v_sb[:], 1.0)

        # ---- phase-scoped SBUF (released after projections) ----
        with tc.tile_pool(name="xp", bufs=1) as xp:
            x_sb = xp.tile([128, CKT * S], bf, name="x_sb")
            wq_sb = xp.tile([128, CKT * DG], bf, name="wq_sb")
            wk_sb = xp.tile([128, CKT * DG], bf, name="wk_sb")
            wv_sb = xp.tile([128, CKT * DG], bf, name="wv_sb")
            bq_sb = xp.tile([128, 1], f32, name="bq_sb")
            bq2_sb = xp.tile([64, 1], f32, name="bq2_sb")
            bk_sb = xp.tile([128, 1], f32, name="bk_sb")
            bk2_sb = xp.tile([64, 1], f32, name="bk2_sb")
            bv_sb = xp.tile([1, DG], f32, name="bv_sb")
            bvb_sb = xp.tile([128, DG], f32, name="bvb_sb")

            for c in range(CKT):
                nc.sync.dma_start(x_sb[:, c * S:(c + 1) * S],
                                  xt_d[c * 128:(c + 1) * 128, :])
                nc.sync.dma_start(wq_sb[:, c * DG:(c + 1) * DG],
                                  wqt_d[c * 128:(c + 1) * 128, :])
                nc.sync.dma_start(wk_sb[:, c * DG:(c + 1) * DG],
                                  wkt_d[c * 128:(c + 1) * 128, :])
                nc.sync.dma_start(wv_sb[:, c * DG:(c + 1) * DG],
                                  wvt_d[c * 128:(c + 1) * 128, :])
            nc.sync.dma_start(bq_sb[:], bq_d[0:128, :])
            nc.sync.dma_start(bq2_sb[:], bq_d[128:DG, :])
            nc.sync.dma_start(bk_sb[:], bk_d[0:128, :])
            nc.sync.dma_start(bk2_sb[:], bk_d[128:DG, :])
            nc.sync.dma_start(bv_sb[:], bv_d[:])
            nc.gpsimd.partition_broadcast(bvb_sb[:], bv_sb[:])

            pj = 0  # alternate the two 1-bank psum tags for double buffering

            def proj_psum(shape, name):
                nonlocal pj
                pj += 1
                return ps.tile(shape, f32, tag=("cA", "cB")[pj % 2], name=name)

            def qk_proj(w_sb, bias, bias2, dst, what):
                # K (and later Q): d-tiles [0:128) = heads 0|1, [128:192) = head 2
                for j in range(NJ):
                    xs = [x_sb[:, c * S + j * 512: c * S + (j + 1) * 512]
                          for c in range(CKT)]
                    p0 = proj_psum([128, 512], f"{what}a{j}")
                    for c in range(CKT):
                        nc.tensor.matmul(p0[:], w_sb[:, c * DG: c * DG + 128],
                                         xs[c], start=(c == 0), stop=(c == CKT - 1))
                    jq = slice(j * 512, (j + 1) * 512)
                    nc.vector.tensor_scalar_add(dst[0][0:64, jq], p0[0:64, :],
                                                bias[0:64, :])
                    nc.vector.tensor_scalar_add(dst[1][64:128, jq], p0[64:128, :],
                                                bias[64:128, :])
                    p1 = proj_psum([64, 512], f"{what}b{j}")
                    for c in range(CKT):
                        nc.tensor.matmul(p1[:], w_sb[:, c * DG + 128: c * DG + 192],
                                         xs[c], start=(c == 0), stop=(c == CKT - 1))
                    nc.vector.tensor_scalar_add(dst[2][0:64, jq], p1[0:64, :],
                                                bias2[0:64, :])
                # duplicate each head's 64 rows into the other partition half
                nc.sync.dma_start(dst[0][64:128, :], dst[0][0:64, :])
                nc.sync.dma_start(dst[1][0:64, :], dst[1][64:128, :])
                nc.sync.dma_start(dst[2][64:128, :], dst[2][0:64, :])

            qk_proj(wk_sb, bk_sb, bk2_sb, kt_, "k")
            qk_proj(wq_sb, bq_sb, bq2_sb, qt, "q")
            if debug_dumps:
                nc.sync.dma_start(dbg["qt0"][:], qt[0][:])
                nc.sync.dma_start(dbg["kt0"][:], kt_[0][:])

            for t in range(NKT):
                pv = proj_psum([128, DG], f"v{t}")
                for c in range(CKT):
                    nc.tensor.matmul(pv[:],
                                     x_sb[:, c * S + t * 128: c * S + (t + 1) * 128],
                                     wv_sb[:, c * DG:(c + 1) * DG],
                                     start=(c == 0), stop=(c == CKT - 1))
                for h in range(NH):
                    nc.vector.tensor_tensor(
                        v_sb[:, t * VW + h * 65: t * VW + h * 65 + 64],
                        pv[:, h * 64:(h + 1) * 64],
                        bvb_sb[:, h * 64:(h + 1) * 64], ALU.add)

        if debug_dumps:
            nc.sync.dma_start(dbg["v"][:], v_sb[:])

        # ---- attention ----
        groups = [list(range(s, min(s + 3, NKT))) for s in range(0, NKT, 3)]

        for j in range(NJ):
            jq = slice(j * 512, (j + 1) * 512)
            for h in range(NH):
                cA = ps.tile([65, 512], f32, tag="cA", name=f"cA{j}_{h}")
                cB = ps.tile([65, 512], f32, tag="cB", name=f"cB{j}_{h}")

                def ctx_mm(t):
                    vs = slice(t * VW + h * 65, t * VW + (h + 1) * 65)
                    es = slice(t * 512, (t + 1) * 512)
                    nc.tensor.matmul(cA[:], v_sb[0:64, vs], expS[0:64, es],
                                     start=(t == 0), stop=(t == NKT - 1))
                    nc.tensor.matmul(cB[:], v_sb[64:128, vs], expS[64:128, es],
                                     start=(t == 0), stop=(t == NKT - 1))

                for gi, kts in enumerate(groups):
                    sc = ps.tile([128, len(kts) * 512], f32, tag="sc", bufs=2,
                                 name=f"sc{j}_{h}_{gi}")
                    for i, t in enumerate(kts):
                        b = 64 * (t % 2)
                        nc.tensor.matmul(
                            sc[:, i * 512:(i + 1) * 512],
                            kt_[h][b:b + 64, t * 128:(t + 1) * 128],
                            qt[h][b:b + 64, jq])
                    nc.scalar.activation(
                        expS[:, kts[0] * 512:(kts[-1] + 1) * 512], sc[:],
                        ACTF.Exp, scale=0.125)
                    if gi > 0:
                        for t in groups[gi - 1]:
                            ctx_mm(t)
                for t in groups[-1]:
                    ctx_mm(t)

                s2 = wp.tile([65, 512], f32, tag="s2", bufs=2, name=f"s2_{j}_{h}")
                sm = wp.tile([1, 512], f32, tag="sm", bufs=2, name=f"sm_{j}_{h}")
                rb = wp.tile([64, 512], f32, tag="rb", bufs=2, name=f"rb_{j}_{h}")
                ri = wp.tile([64, 512], f32, tag="ri", bufs=2, name=f"ri_{j}_{h}")
                nc.vector.tensor_copy(s2[:], cA[:])
                nc.vector.tensor_tensor(s2[:], s2[:], cB[:], ALU.add)
                # partition_broadcast on HW reads the tile's physical
                # partition 0, so stage the sums row into a base-0 tile first
                nc.sync.dma_start(sm[:], s2[64:65, :])
                nc.gpsimd.partition_broadcast(rb[:], sm[:])
                nc.vector.reciprocal(ri[:], rb[:])
                nc.vector.tensor_tensor(ctxN[h][:, jq], s2[0:64, :], ri[:],
                                        ALU.mult)
                if debug_dumps and j == 0 and h == 0:
                    nc.sync.dma_start(dbg["expS"][:], expS[:])
                    nc.sync.dma_start(dbg["s2"][:], s2[:])
                    nc.sync.dma_start(dbg["ri"][:], ri[:])

        if debug_dumps:
            nc.sync.dma_start(dbg["ctxN0"][:], ctxN[0][:])

        # ---- output projection ----
        for qs in range(S // 128):
            po = ps.tile([128, D], f32, tag="sc", bufs=2, name=f"po{qs}")
            qq = slice(qs * 128, (qs + 1) * 128)
            for c0, cw in ((0, 512), (512, 256)):
                for h in range(NH):
                    nc.tensor.matmul(po[:, c0:c0 + cw], ctxN[h][:, qq],
                                     wo_sb[h][:, c0:c0 + cw],
                                     start=(h == 0), stop=(h == NH - 1))
            ob = wp.tile([128, D], f32, tag="ob", bufs=2, name=f"ob{qs}")
            nc.vector.tensor_copy(ob[:], po[:])
            nc.sync.dma_start(out_d[qq, :], ob[:])

    nc.compile()
    return nc


_CACHE = {}


def get_nc(S=4096):
    if S not in _CACHE:
        _CACHE[S] = build_nc(S)
    return _CACHE[S]


def make_in_maps(x, Wq, bq, Wk, bk, Wv, bv, Wo):
    f32 = np.float32
    in_maps = []
    for c in range(8):
        b, g = c // 4, c % 4
        hs = slice(g * DG, (g + 1) * DG)
        in_maps.append({
            "xt": np.ascontiguousarray(x[b].T).astype(BF16),
            "wqt": np.ascontiguousarray(Wq[hs].T).astype(BF16),
            "wkt": np.ascontiguousarray(Wk[hs].T).astype(BF16),
            "wvt": np.ascontiguousarray(Wv[hs].T).astype(BF16),
            "wot": np.ascontiguousarray(Wo[:, hs].T).astype(BF16),
            "bq": bq[hs].reshape(DG, 1).astype(f32),
            "bk": bk[hs].reshape(DG, 1).astype(f32),
            "bv": bv[hs].reshape(1, DG).astype(f32),
        })
    return in_maps


def kernel(x, Wq, bq, Wk, bk, Wv, bv, Wo, bo):
    from concourse import bass_utils

    x = np.asarray(x)
    B, S, _ = x.shape
    nc = get_nc(S)
    in_maps = make_in_maps(np.asarray(x), np.asarray(Wq), np.asarray(bq),
                           np.asarray(Wk), np.asarray(bk), np.asarray(Wv),
                           np.asarray(bv), np.asarray(Wo))
    res = bass_utils.run_bass_kernel_spmd(nc, in_maps, core_ids=list(range(8)))
    out = np.zeros((B, S, D), np.float32)
    for c in range(8):
        out[c // 4] += res.results[c]["out"]
    out += np.asarray(bo).astype(np.float32)[None, None, :]
    return out



# revision 4
# speedup vs baseline: 1.0104x; 1.0104x over previous
"""Trainium2 Bass kernel: 12-head MHA (B=2, S=4096, D=768), sharded over 8 cores.

Sharding: core c -> batch b = c//4, head group g = c%4 (3 heads of 64 dims each).
Each core computes a partial output [S, 768] = ctx_g @ Wo_g^T for its 3 heads;
the host sums the 4 partials per batch and adds bo.

v2: the kernel is organized as one flat software pipeline over the 264
(q-chunk j, head h, k-tile group gi) exp groups so the ScalarE exp stream --
the structural bottleneck at ~1.5us per [128,1536] group -- runs continuously
from ~25us after start. Per flat iteration: exp(gi) runs while the PE does
scores(gi+1) and context(gi-1); V/Q projections and the output projection are
threaded through the spare PSUM bank ("pp" tag) as side tasks hidden under the
exp stream. Context uses a single K=128 accumulator [65,512] (V_aug ones row
gives the softmax denominator). PSUM: scores 2x3 banks + ctx 1 + pp 1 = 8.
"""

import sys

for _p in ("/opt/trn_rl_repo",):
    if _p not in sys.path:
        sys.path.insert(0, _p)

import numpy as np
import ml_dtypes

BF16 = ml_dtypes.bfloat16

D = 768
DG = 192          # head dims per core (3 heads x 64)
DK = 64
NH = 3            # heads per core
CKT = D // 128    # contraction k-tiles for projections


def build_nc(S):
    import concourse.mybir as mybir
    import concourse.tile as tile
    import concourse.bacc as bacc
    from contextlib import ExitStack

    dt = mybir.dt
    f32, bf = dt.float32, dt.bfloat16
    ALU = mybir.AluOpType
    ACTF = mybir.ActivationFunctionType

    NKT = S // 128        # seq k-tiles
    NJ = S // 512         # q chunks
    VW = NH * 65          # V_aug cols per seq tile
    GROUPS = [list(range(s, min(s + 3, NKT))) for s in range(0, NKT, 3)]
    NG = len(GROUPS)

    nc = bacc.Bacc("TRN2", target_bir_lowering=False, debug=False)

    xt_d = nc.dram_tensor("xt", [D, S], bf, kind="ExternalInput").ap()
    wqt_d = nc.dram_tensor("wqt", [D, DG], bf, kind="ExternalInput").ap()
    wkt_d = nc.dram_tensor("wkt", [D, DG], bf, kind="ExternalInput").ap()
    wvt_d = nc.dram_tensor("wvt", [D, DG], bf, kind="ExternalInput").ap()
    wot_d = nc.dram_tensor("wot", [DG, D], bf, kind="ExternalInput").ap()
    bq_d = nc.dram_tensor("bq", [DG, 1], f32, kind="ExternalInput").ap()
    bk_d = nc.dram_tensor("bk", [DG, 1], f32, kind="ExternalInput").ap()
    bv_d = nc.dram_tensor("bv", [1, DG], f32, kind="ExternalInput").ap()
    out_d = nc.dram_tensor("out", [S, D], f32, kind="ExternalOutput").ap()

    with tile.TileContext(nc) as tc, ExitStack() as ctx:
        wp = ctx.enter_context(tc.tile_pool(name="wp", bufs=1))
        ps = ctx.enter_context(tc.tile_pool(name="ps", bufs=1, space="PSUM"))

        # ---- persistent SBUF ----
        qt = [wp.tile([128, S], bf, name=f"qt{h}") for h in range(NH)]
        kt_ = [wp.tile([128, S], bf, name=f"kt{h}") for h in range(NH)]
        v_sb = wp.tile([128, NKT * VW], bf, name="v_sb")
        expS = wp.tile([128, NKT * 512], bf, name="expS")
        # heads 0|1 share a [128, S] tile so out-proj lhsT/rhs base partitions
        # line up with the matching Wo tiles; head 2 gets rows 0:64 of its own.
        ctxN01 = wp.tile([128, S], bf, name="ctxN01")
        ctxN2 = wp.tile([128, S], bf, name="ctxN2")
        wo01 = wp.tile([128, D], bf, name="wo01")
        wo2 = wp.tile([128, D], bf, name="wo2")
        x_sb = wp.tile([128, CKT * S], bf, name="x_sb")
        wq_sb = wp.tile([128, CKT * DG], bf, name="wq_sb")
        wk_sb = wp.tile([128, CKT * DG], bf, name="wk_sb")
        wv_sb = wp.tile([128, CKT * DG], bf, name="wv_sb")
        bq_sb = wp.tile([128, 1], f32, name="bq_sb")
        bq2_sb = wp.tile([64, 1], f32, name="bq2_sb")
        bk_sb = wp.tile([128, 1], f32, name="bk_sb")
        bk2_sb = wp.tile([64, 1], f32, name="bk2_sb")
        bv_sb = wp.tile([1, DG], f32, name="bv_sb")
        bvb_sb = wp.tile([128, DG], f32, name="bvb_sb")
        warm_sb = wp.tile([128, 1], f32, name="warm_sb")

        def ctxN(h):
            return (ctxN01[0:64, :], ctxN01[64:128, :], ctxN2[0:64, :])[h]

        def wo(h):
            return (wo01[0:64, :], wo01[64:128, :], wo2[0:64, :])[h]

        # ---- input DMA: weights first, then x in j-major [128,512] blocks ----
        for c in range(CKT):
            nc.sync.dma_start(wk_sb[:, c * DG:(c + 1) * DG],
                              wkt_d[c * 128:(c + 1) * 128, :])
        for c in range(CKT):
            nc.sync.dma_start(wq_sb[:, c * DG:(c + 1) * DG],
                              wqt_d[c * 128:(c + 1) * 128, :])
        for c in range(CKT):
            nc.sync.dma_start(wv_sb[:, c * DG:(c + 1) * DG],
                              wvt_d[c * 128:(c + 1) * 128, :])
        nc.sync.dma_start(wo01[0:64, :], wot_d[0:64, :])
        nc.sync.dma_start(wo01[64:128, :], wot_d[64:128, :])
        nc.sync.dma_start(wo2[0:64, :], wot_d[128:192, :])
        nc.sync.dma_start(bq_sb[:], bq_d[0:128, :])
        nc.sync.dma_start(bq2_sb[:], bq_d[128:DG, :])
        nc.sync.dma_start(bk_sb[:], bk_d[0:128, :])
        nc.sync.dma_start(bk2_sb[:], bk_d[128:DG, :])
        nc.sync.dma_start(bv_sb[:], bv_d[:])
        nc.gpsimd.partition_broadcast(bvb_sb[:], bv_sb[:])
        # V_aug ones columns: set whole tile to 1.0, data cols overwritten later
        nc.gpsimd.memset(v_sb[:], 1.0)
        for j in range(NJ):
            for c in range(CKT):
                nc.sync.dma_start(x_sb[:, c * S + j * 512: c * S + (j + 1) * 512],
                                  xt_d[c * 128:(c + 1) * 128, j * 512:(j + 1) * 512])

        # ---- Q/K projection for one q-chunk j ----
        # psum layout: [128, 0:512] = heads 0|1 (M=128), [0:64, 512:1024] = head 2
        def qk_proj_j(w_sb, bias, bias2, dst, j, tag, bufs, what):
            jq = slice(j * 512, (j + 1) * 512)
            xs = [x_sb[:, c * S + j * 512: c * S + (j + 1) * 512]
                  for c in range(CKT)]
            p0 = ps.tile([128, 512], f32, tag=tag, bufs=bufs, name=f"{what}a{j}")
            for c in range(CKT):
                nc.tensor.matmul(p0[:], w_sb[:, c * DG: c * DG + 128],
                                 xs[c], start=(c == 0), stop=(c == CKT - 1))
            nc.vector.tensor_scalar_add(dst[0][0:64, jq], p0[0:64, :],
                                        bias[0:64, :])
            nc.vector.tensor_scalar_add(dst[1][64:128, jq], p0[64:128, :],
                                        bias[64:128, :])
            p1 = ps.tile([64, 512], f32, tag=tag, bufs=bufs, name=f"{what}b{j}")
            for c in range(CKT):
                nc.tensor.matmul(p1[:], w_sb[:, c * DG + 128: c * DG + 192],
                                 xs[c], start=(c == 0), stop=(c == CKT - 1))
            nc.vector.tensor_scalar_add(dst[2][0:64, jq], p1[0:64, :],
                                        bias2[0:64, :])
            # duplicate each head's 64 rows into the other partition half so
            # score matmuls can sit at tile_position rows 0/64 by k-tile parity
            nc.sync.dma_start(dst[0][64:128, jq], dst[0][0:64, jq])
            nc.sync.dma_start(dst[1][0:64, jq], dst[1][64:128, jq])
            nc.sync.dma_start(dst[2][64:128, jq], dst[2][0:64, jq])

        # K projection for all j upfront (through the sc rotation -- scores
        # haven't started, so the 2x3-bank slots double-buffer the drains),
        # then Q for j=0 only; Q for j>=1 is a side task under the exp stream.
        for j in range(NJ):
            qk_proj_j(wk_sb, bk_sb, bk2_sb, kt_, j, "sc", 2, "k")
        qk_proj_j(wq_sb, bq_sb, bq2_sb, qt, 0, "sc", 2, "q")

        # ---- pipeline stage emitters ----
        def s_group(j, h, gi):
            kts = GROUPS[gi]
            jq = slice(j * 512, (j + 1) * 512)
            sc = ps.tile([128, len(kts) * 512], f32, tag="sc", bufs=2,
                         name=f"sc{j}_{h}_{gi}")
            for i, t in enumerate(kts):
                b = 64 * (t % 2)
                nc.tensor.matmul(sc[:, i * 512:(i + 1) * 512],
                                 kt_[h][b:b + 64, t * 128:(t + 1) * 128],
                                 qt[h][b:b + 64, jq])
            return sc

        def exp_group(j, h, gi, sc):
            kts = GROUPS[gi]
            nc.scalar.activation(expS[:, kts[0] * 512:(kts[-1] + 1) * 512],
                                 sc[:], ACTF.Exp, scale=0.125)

        def ctx_group(h, gi, ct):
            for t in GROUPS[gi]:
                nc.tensor.matmul(ct[:],
                                 v_sb[:, t * VW + h * 65: t * VW + h * 65 + 65],
                                 expS[:, t * 512:(t + 1) * 512],
                                 start=(t == 0), stop=(t == NKT - 1))

        def norm(j, h, ct):
            jq = slice(j * 512, (j + 1) * 512)
            s2 = wp.tile([65, 512], f32, tag="s2", bufs=2, name=f"s2_{j}_{h}")
            sm = wp.tile([1, 512], f32, tag="sm", bufs=2, name=f"sm_{j}_{h}")
            rb = wp.tile([64, 512], f32, tag="rb", bufs=2, name=f"rb_{j}_{h}")
            ri = wp.tile([64, 512], f32, tag="ri", bufs=2, name=f"ri_{j}_{h}")
            nc.vector.tensor_copy(s2[:], ct[:])
            # partition_broadcast reads physical partition 0: stage the
            # denominator row into a base-0 tile first
            nc.sync.dma_start(sm[:], s2[64:65, :])
            nc.gpsimd.partition_broadcast(rb[:], sm[:])
            nc.vector.reciprocal(ri[:], rb[:])
            nc.vector.tensor_tensor(ctxN(h)[:, jq], s2[0:64, :], ri[:],
                                    ALU.mult)

        def v_tile(t):
            pv = ps.tile([128, DG], f32, tag="pp", bufs=1, name=f"pv{t}")
            for c in range(CKT):
                nc.tensor.matmul(pv[:],
                                 x_sb[:, c * S + t * 128: c * S + (t + 1) * 128],
                                 wv_sb[:, c * DG:(c + 1) * DG],
                                 start=(c == 0), stop=(c == CKT - 1))
            for h in range(NH):
                nc.vector.tensor_tensor(
                    v_sb[:, t * VW + h * 65: t * VW + h * 65 + 64],
                    pv[:, h * 64:(h + 1) * 64],
                    bvb_sb[:, h * 64:(h + 1) * 64], ALU.add)

        ob = {}

        def po_chunk(j, qs, half):
            c0 = half * 384
            pq = slice(qs * 128, (qs + 1) * 128)
            if half == 0:
                ob[qs % 2] = wp.tile([128, D], f32, tag=f"ob{qs % 2}", bufs=2,
                                     name=f"ob{j}_{qs}")
            po = ps.tile([128, 384], f32, tag="pp", bufs=1,
                         name=f"po{j}_{qs}_{half}")
            # heads 0|1 packed as one K=128 matmul; all matmuls of the group
            # keep tile_position (0,0) -- mixed row positions in one
            # accumulation group break the HW path
            nc.tensor.matmul(po[:], ctxN01[:, pq], wo01[:, c0:c0 + 384],
                             start=True, stop=False)
            nc.tensor.matmul(po[:], ctxN2[0:64, pq], wo2[0:64, c0:c0 + 384],
                             start=False, stop=True)
            nc.vector.tensor_copy(ob[qs % 2][:, c0:c0 + 384], po[:])
            if half == 1:
                nc.sync.dma_start(out_d[pq, :], ob[qs % 2][:])

        # ---- side-task schedule keyed by flat iteration index ----
        # j0/h0 iterations carry the V projection (3 tiles per group, just in
        # time for ctx); (j,h1) iterations carry the Q projection for j+1;
        # the 8 out-proj chunks of j are emitted in the first 4 iterations of
        # j+1 (after norm(j,2), which happens at the j-boundary iteration).
        side = {}

        def add_side(idx, fn):
            side.setdefault(idx, []).append(fn)

        for gi in range(NG):
            for t in GROUPS[gi]:
                add_side(gi, (lambda t=t: v_tile(t)))
        for j in range(NJ - 1):
            base = j * NH * NG + NG  # start of (j, h=1)
            add_side(base + 2, (lambda j=j: qk_proj_j(
                wq_sb, bq_sb, bq2_sb, qt, j + 1, "pp", 1, "q")))

        def add_po(j, idx0):
            for qs4 in range(4):
                for half in range(2):
                    add_side(idx0 + qs4,
                             (lambda j=j, qs=j * 4 + qs4, half=half:
                              po_chunk(j, qs, half)))

        # ---- the flat pipeline ----
        flat = [(j, h, gi) for j in range(NJ) for h in range(NH)
                for gi in range(NG)]
        sc_t = {0: s_group(*flat[0])}
        ct_t = None
        for idx, (j, h, gi) in enumerate(flat):
            exp_group(j, h, gi, sc_t.pop(idx))
            if idx + 1 < len(flat):
                sc_t[idx + 1] = s_group(*flat[idx + 1])
            if idx > 0:
                pj, ph, pgi = flat[idx - 1]
                if pgi == 0:
                    ct_t = ps.tile([65, 512], f32, tag="ct", bufs=1,
                                   name=f"ct{pj}_{ph}")
                ctx_group(ph, pgi, ct_t)
                if pgi == NG - 1:
                    norm(pj, ph, ct_t)
                    if ph == NH - 1:
                        add_po(pj, idx)
            for fn in side.pop(idx, ()):
                fn()
        # epilogue: last group's ctx + norm + out-proj for j = NJ-1
        pj, ph, pgi = flat[-1]
        ctx_group(ph, pgi, ct_t)
        norm(pj, ph, ct_t)
        for qs4 in range(4):
            for half in range(2):
                po_chunk(pj, pj * 4 + qs4, half)

    nc.compile()
    return nc


_CACHE = {}


def get_nc(S=4096):
    if S not in _CACHE:
        _CACHE[S] = build_nc(S)
    return _CACHE[S]


def make_in_maps(x, Wq, bq, Wk, bk, Wv, bv, Wo):
    f32 = np.float32
    in_maps = []
    for c in range(8):
        b, g = c // 4, c % 4
        hs = slice(g * DG, (g + 1) * DG)
        in_maps.append({
            "xt": np.ascontiguousarray(x[b].T).astype(BF16),
            "wqt": np.ascontiguousarray(Wq[hs].T).astype(BF16),
            "wkt": np.ascontiguousarray(Wk[hs].T).astype(BF16),
            "wvt": np.ascontiguousarray(Wv[hs].T).astype(BF16),
            "wot": np.ascontiguousarray(Wo[:, hs].T).astype(BF16),
            "bq": bq[hs].reshape(DG, 1).astype(f32),
            "bk": bk[hs].reshape(DG, 1).astype(f32),
            "bv": bv[hs].reshape(1, DG).astype(f32),
        })
    return in_maps


def kernel(x, Wq, bq, Wk, bk, Wv, bv, Wo, bo):
    from concourse import bass_utils

    x = np.asarray(x)
    B, S, _ = x.shape
    nc = get_nc(S)
    in_maps = make_in_maps(np.asarray(x), np.asarray(Wq), np.asarray(bq),
                           np.asarray(Wk), np.asarray(bk), np.asarray(Wv),
                           np.asarray(bv), np.asarray(Wo))
    res = bass_utils.run_bass_kernel_spmd(nc, in_maps, core_ids=list(range(8)))
    out = np.zeros((B, S, D), np.float32)
    for c in range(8):
        out[c // 4] += res.results[c]["out"]
    out += np.asarray(bo).astype(np.float32)[None, None, :]
    return out
